# revision 12
# baseline (speedup 1.0000x reference)
"""Trainium2 Bass kernel for nn_MessagePassing_300647711374.

Sharding: 8 cores = 2 batches x 4 D-chunks of 32 planes, 3-plane halo
supplied by the host (zero outside global domain). Each core computes the
12-conv message-passing chain on its extended chunk and returns per-channel
*deltas* (conv contributions) on its 32 interior planes; the host adds
deltas to the input feature and recomputes the few global-D-boundary planes
of the chained channels exactly (reference zero-pads intermediates at the
domain edge, which a halo'd chain cannot reproduce).

Device algorithm per 3x3x3 conv: H on partitions, columns = (d, w) with a
1-col zero pad per 129-col plane. For each (dz,dw) of 9, a tri-diagonal
128x128 band matrix (dy taps) multiplies a shifted column slice, all 9
accumulating in PSUM. dtype float32r (TF32-like, ~4 cols/cycle on PE).
"""
import numpy as np

import concourse.bass as bass
import concourse.tile as tile
from concourse import bacc, mybir
from concourse.bass2jax import _bass_exec_p, install_neuronx_cc_hook, partition_id_tensor

P = 128
PL = 129                     # 1 pad col + 128 w cols
G = 1                        # leading guard col
D_CHUNK = 32
HALO = 3
EXT = D_CHUNK + 2 * HALO     # 38
W_BUF = G + EXT * PL + 6
I0, I1 = HALO, HALO + D_CHUNK

CH_IN = [4, 5, 6, 7, 8, 10]
CH_OUT = [0, 1, 2, 3, 5, 7, 8, 10, 12, 13]
N_CORES = 8

# (name, band_idx, src) groups; each entry is one psum chain with sinks
# pair entries accumulate two convs into the same psum.
# Order: 5 independent convs (phase 1), then the dependent chain.
# Intermediates are written IN-PLACE into their base buffers (reference
# ordering reads the original values only before each update; Tile's WAR
# tracking serializes reads-before-overwrite).
SCHED = [
    (("conv02", 1, "x4"), ("conv01", 0, "x5"), (I0, I1), dict(delta=0)),
    (("conv50", 2, "x4"), None, (I0 - 1, I1 + 1), dict(delta=5, inter=("f5n", "x5"))),
    (("conv10", 4, "x7"), ("conv11", 5, "x6"), (I0, I1), dict(delta=1)),
    (("conv70", 6, "x6"), None, (I0 - 1, I1 + 1), dict(delta=7, inter=("f7n", "x7"))),
    (("conv80", 8, "x10"), None, (I0 - 2, I1 + 2), dict(delta=8, inter=("f8n", "x8"))),
    (("conv100", 9, "f8n"), None, (I0 - 1, I1 + 1), dict(delta=10, inter=("f10n", "x10"))),
    (("conv20", 3, "f5n"), None, (I0, I1), dict(delta=2)),
    (("conv30", 7, "f7n"), None, (I0, I1), dict(delta=3)),
    (("conv120", 10, "f8n"), None, (I0, I1), dict(delta=12)),
    (("conv130", 11, "f10n"), None, (I0, I1), dict(delta=13)),
]


def groups_for(a, b):
    out, p, rem = [], a, b - a
    while rem >= 5:
        out.append((p, 3)); p += 3; rem -= 3
    if rem == 4:
        out.append((p, 2)); out.append((p + 2, 2))
    elif rem > 0:
        out.append((p, rem))
    return out


# ---------------------------------------------------------------- device ---
def build_nc(reps=1, ablate=()):
    """ablate: subset of {'in_dma','mm','adds','out'} to skip (timing probes)."""
    from contextlib import ExitStack
    f32 = mybir.dt.float32
    f32r = mybir.dt.float32r

    nc = bacc.Bacc("TRN2", target_bir_lowering=False, debug=False,
                   num_devices=N_CORES)
    xin = nc.dram_tensor("xin", [6, P, EXT, P], f32r, kind="ExternalInput").ap()
    bands = nc.dram_tensor("bands", [12, P, 9 * P], f32r, kind="ExternalInput").ap()
    deltas = nc.dram_tensor("deltas", [10, P, D_CHUNK, P], f32,
                            kind="ExternalOutput").ap()
    dmap = {c: i for i, c in enumerate(CH_OUT)}

    with tile.TileContext(nc) as tc:
        with ExitStack() as ctx:
            chan = ctx.enter_context(tc.tile_pool(name="chan", bufs=6))
            bpool = ctx.enter_context(tc.tile_pool(name="bands", bufs=1))
            stage = ctx.enter_context(tc.tile_pool(name="stage", bufs=6))
            psum = ctx.enter_context(tc.tile_pool(name="psum", bufs=8, space="PSUM"))

            def body(_it):
                bufs = {}

                def new_chan(name):
                    t = chan.tile([P, W_BUF], f32r, tag="chan")
                    # zero pads: per-plane pos-0 cols, leading guard, tail
                    pads = t[:, G:G + EXT * PL].rearrange(
                        "p (d c) -> p d c", c=PL)[:, :, 0:1]
                    nc.vector.memset(pads.bitcast(f32), 0.0)
                    nc.vector.memset(t[:, 0:G].bitcast(f32), 0.0)
                    nc.vector.memset(t[:, G + EXT * PL:W_BUF].bitcast(f32), 0.0)
                    bufs[name] = t
                    return t

                def load_chan(name, k):
                    t = new_chan(name)
                    if 'in_dma' in ablate:
                        return
                    spl = (0, 5, 10, 15, 20, 25, 30, 34, EXT)
                    for (a, b) in zip(spl[:-1], spl[1:]):
                        dst = t[:, G + a * PL:G + b * PL].rearrange(
                            "p (d c) -> p d c", c=PL)[:, :, 1:129]
                        nc.sync.dma_start(dst, xin[k, :, a:b, :])

                ball = bpool.tile([P, 12 * 9 * P], f32r, tag="band")
                for ci in range(12):
                    nc.sync.dma_start(ball[:, ci * 9 * P:(ci + 1) * 9 * P], bands[ci])
                btiles = [ball[:, ci * 9 * P:(ci + 1) * 9 * P] for ci in range(12)]

                load_chan("x4", 0)
                load_chan("x5", 1)
                load_chan("x6", 2)
                load_chan("x7", 3)
                load_chan("x10", 5)
                load_chan("x8", 4)

                def run_conv(main, second, rng, sinks):
                    a, b = rng
                    for (p0, npl) in groups_for(a, b):
                        NB = npl * PL + (npl * PL) % 2
                        acc = psum.tile([P, 388], f32, tag="psum")
                        convs = [main] + ([second] if second else [])
                        n_mm = 9 * len(convs)
                        mm = 0
                        for (_nm, ci, src) in convs:
                            xb = bufs[src]
                            for dz in (-1, 0, 1):
                                for dw in (-1, 0, 1):
                                    j = (dz + 1) * 3 + (dw + 1)
                                    s0 = G + (p0 + dz) * PL + dw
                                    assert 0 <= s0 and s0 + NB <= W_BUF, (p0, dz, dw)
                                    if 'mm' in ablate:
                                        mm += 1
                                        continue
                                    nc.tensor.matmul(
                                        acc[:, 0:NB],
                                        btiles[ci][:, j * P:(j + 1) * P],
                                        xb[:, s0:s0 + NB],
                                        start=(mm == 0), stop=(mm == n_mm - 1))
                                    mm += 1
                        acc3 = acc[:, 0:npl * PL].rearrange(
                            "p (d c) -> p d c", c=PL)[:, :, 1:129]
                        if "inter" in sinks:
                            # intermediate IS the final channel value:
                            # DVE add psum+base -> buffer, DMA interior later
                            dst_name, base = sinks["inter"]
                            if dst_name not in bufs:
                                bufs[dst_name] = bufs[base]  # in-place update
                            dcols = bufs[dst_name][:, G + p0 * PL:G + (p0 + npl) * PL
                                                   ].rearrange("p (d c) -> p d c",
                                                               c=PL)[:, :, 1:129]
                            bcols = bufs[base][:, G + p0 * PL:G + (p0 + npl) * PL
                                               ].rearrange("p (d c) -> p d c",
                                                           c=PL)[:, :, 1:129]
                            if 'adds' not in ablate:
                                nc.vector.tensor_add(dcols, acc3, bcols)
                            k = dmap[sinks["delta"]]
                            lo, hi = max(p0, I0), min(p0 + npl, I1)
                            if lo < hi and 'out' not in ablate:
                                src3 = dcols[:, lo - p0:hi - p0, :].bitcast(f32)
                                dst = deltas[k, :, lo - I0:hi - I0, :]
                                nc.sync.dma_start(dst, src3)
                        else:
                            # pure delta: PSUM -> SBUF staging (ACT) -> DMA
                            k = dmap[sinks["delta"]]
                            lo, hi = max(p0, I0), min(p0 + npl, I1)
                            if lo < hi and 'out' not in ablate:
                                st = stage.tile([P, 3 * P], f32, tag="stage")
                                npv = hi - lo
                                st3 = st[:, 0:npv * P].rearrange(
                                    "p (d c) -> p d c", c=P)
                                nc.scalar.mul(st3, acc3[:, lo - p0:hi - p0, :], 1.0)
                                dst = deltas[k, :, lo - I0:hi - I0, :]
                                nc.sync.dma_start(dst, st3)

                for i, (main, second, rng, sinks) in enumerate(SCHED):
                    run_conv(main, second, rng, sinks)

            if reps > 1:
                with tc.For_i(0, reps, 1) as it:
                    body(it)
            else:
                body(0)
    nc.compile()
    return nc


# ------------------------------------------------------------------ host ---
def build_bands(weights):
    bands = np.zeros((12, P, 9 * P), dtype=np.float32)
    eyes = {d: np.eye(P, k=-d, dtype=np.float32) for d in (-1, 0, 1)}
    for ci in range(12):
        for dz in (-1, 0, 1):
            for dw in (-1, 0, 1):
                j = (dz + 1) * 3 + (dw + 1)
                band = sum(weights[ci, dz + 1, dy + 1, dw + 1] * eyes[dy]
                           for dy in (-1, 0, 1))
                bands[ci, :, j * P:(j + 1) * P] = band
    return bands


def make_shards(feature):
    shards = []
    for c in range(N_CORES):
        b, q = divmod(c, 4)
        d0 = q * D_CHUNK - HALO
        xin = np.zeros((6, P, EXT, P), dtype=np.float32)
        lo, hi = max(d0, 0), min(d0 + EXT, 128)
        for k, ch in enumerate(CH_IN):
            xin[k, :, lo - d0:hi - d0, :] = feature[b, ch, lo:hi].transpose(1, 0, 2)
        shards.append(xin)
    return shards


def _shift2(pl, dy, dw):
    out = np.zeros_like(pl)
    out[max(-dy, 0):P + min(-dy, 0), max(-dw, 0):P + min(-dw, 0)] = \
        pl[max(dy, 0):P + min(dy, 0), max(dw, 0):P + min(dw, 0)]
    return out


def _cp(getter, wk, d):
    acc = np.zeros((P, P), np.float32)
    for dz in (-1, 0, 1):
        p = d + dz
        if not 0 <= p < P:
            continue
        pl = getter(p)
        for dy in (-1, 0, 1):
            for dw in (-1, 0, 1):
                acc += wk[dz + 1, dy + 1, dw + 1] * _shift2(pl, dy, dw)
    return acc


def fix_boundaries(out, feature, weights):
    """Recompute global-D-edge planes of chained channels with exact
    reference semantics (intermediates zeroed outside the domain)."""
    DFIX = [0, 1, 126, 127]
    w50, w20, w70, w30 = weights[2], weights[3], weights[6], weights[7]
    w80, w100, w120, w130 = weights[8], weights[9], weights[10], weights[11]
    for b in range(feature.shape[0]):
        f = feature[b]

        def cache(fn):
            c = {}
            def g(p):
                if p not in c:
                    c[p] = fn(p)
                return c[p]
            return g

        f5n = cache(lambda p: f[5][p] + _cp(lambda q: f[4][q], w50, p))
        f7n = cache(lambda p: f[7][p] + _cp(lambda q: f[6][q], w70, p))
        f8n = cache(lambda p: f[8][p] + _cp(lambda q: f[10][q], w80, p))
        f10n = cache(lambda p: f[10][p] + _cp(f8n, w100, p))
        for d in DFIX:
            out[b, 2, d] = f[2][d] + _cp(f5n, w20, d)
            out[b, 3, d] = f[3][d] + _cp(f7n, w30, d)
            out[b, 10, d] = f[10][d] + _cp(f8n, w100, d)
            out[b, 12, d] = f[12][d] + _cp(f8n, w120, d)
            out[b, 13, d] = f[13][d] + _cp(f10n, w130, d)


# ------------------------------------------------------- runner (cached) ---
_RUNNER = None


def _make_runner(nc):
    import jax
    from jax.sharding import Mesh, PartitionSpec, NamedSharding
    from jax.experimental.shard_map import shard_map

    install_neuronx_cc_hook()
    partition_name = nc.partition_id_tensor.name if nc.partition_id_tensor else None
    in_names, out_names, out_avals, zero_outs = [], [], [], []
    for alloc in nc.m.functions[0].allocations:
        if not isinstance(alloc, mybir.MemoryLocationSet):
            continue
        name = alloc.memorylocations[0].name
        if alloc.kind == "ExternalInput":
            if name != partition_name:
                in_names.append(name)
        elif alloc.kind == "ExternalOutput":
            out_names.append(name)
            shape = tuple(alloc.tensor_shape)
            dtype = mybir.dt.np(alloc.dtype)
            out_avals.append(jax.core.ShapedArray(shape, dtype))
            zero_outs.append(np.zeros(shape, dtype))
    n_params, n_outs = len(in_names), len(out_avals)
    all_in = list(in_names) + list(out_names)
    if partition_name is not None:
        all_in.append(partition_name)

    def _body(*args):
        operands = list(args)
        if partition_name is not None:
            operands.append(partition_id_tensor())
        return tuple(_bass_exec_p.bind(
            *operands, out_avals=tuple(out_avals), in_names=tuple(all_in),
            out_names=tuple(out_names),
            lowering_input_output_aliases=(),
            sim_require_finite=True, sim_require_nnan=True, nc=nc))

    devices = jax.devices()[:N_CORES]
    mesh = Mesh(np.asarray(devices), ("core",))
    sharded = jax.jit(
        shard_map(_body, mesh=mesh,
                  in_specs=(PartitionSpec("core"),) * (n_params + n_outs),
                  out_specs=(PartitionSpec("core"),) * n_outs,
                  check_rep=False),
        keep_unused=True)
    sharding = NamedSharding(mesh, PartitionSpec("core"))
    concat_zeros = [
        jax.device_put(np.zeros((N_CORES * z.shape[0], *z.shape[1:]), z.dtype),
                       sharding)
        for z in zero_outs]

    import jax as _jax

    def prepare(per_core_inputs):
        return [
            _jax.device_put(
                np.concatenate([np.asarray(m[n]) for m in per_core_inputs], axis=0),
                sharding)
            for n in in_names]

    def exec_dev(concat_in):
        return sharded(*concat_in, *concat_zeros)

    def run(per_core_inputs):
        outs = exec_dev(prepare(per_core_inputs))
        outs = [np.asarray(o) for o in outs]
        return [
            {n: outs[i].reshape(N_CORES, *out_avals[i].shape)[c]
             for i, n in enumerate(out_names)}
            for c in range(N_CORES)]

    run.prepare = prepare
    run.exec_dev = exec_dev
    return run


def get_runner():
    global _RUNNER
    if _RUNNER is None:
        nc = build_nc(reps=1)
        _RUNNER = _make_runner(nc)
    return _RUNNER


# ------------------------------------------------------------- entrypoint ---
def kernel(feature, weights):
    feature = np.ascontiguousarray(np.asarray(feature, dtype=np.float32))
    weights = np.ascontiguousarray(np.asarray(weights, dtype=np.float32))
    run = get_runner()
    bands = build_bands(weights)
    shards = make_shards(feature)
    in_maps = [{"xin": x, "bands": bands} for x in shards]
    results = run(in_maps)

    out = np.array(feature, copy=True)
    VAL_SET = {5, 7, 8, 10}  # channels returned as full values, not deltas
    for c in range(N_CORES):
        b, q = divmod(c, 4)
        d0 = q * D_CHUNK
        d = results[c]["deltas"]
        for k, ch in enumerate(CH_OUT):
            if ch in VAL_SET:
                out[b, ch, d0:d0 + D_CHUNK] = d[k].transpose(1, 0, 2)
            else:
                out[b, ch, d0:d0 + D_CHUNK] += d[k].transpose(1, 0, 2)
    fix_boundaries(out, feature, weights)
    return out


if __name__ == "__main__":
    rng = np.random.default_rng(0)
    feature = rng.standard_normal((2, 17, 128, 128, 128), dtype=np.float32)
    weights = (rng.standard_normal((12, 3, 3, 3)) * 0.1).astype(np.float32)
    out = kernel(feature, weights)
    print("kernel ran, out shape", out.shape, out.dtype)
